# revision 1
# baseline (speedup 1.0000x reference)
"""Trainium2 Bass kernel for nn_CrossLayer (B=8, C=256, S=2048, D=64).

Reference computation (per batch b):
    scores = b_i @ c_i^T               [S, S]
    mid    = softmax(scores, axis=-1)  row softmax over m
    out    = a_i @ mid^T + a_i         [C, S]

Strategy: data-parallel over batch — one batch per NeuronCore (8 cores).
Everything is computed in the "column" layout scoresT[m, n] so the softmax
contraction axis m sits on SBUF partitions, which is what the second matmul
needs.  The n axis is split into two halves of 1024 so PSUM can hold, at
the same time, the scores tile for the current m-tile AND the out2
accumulators, letting phase-2 matmuls interleave with phase-1 per m-tile:

  per (half h, m-tile mt):
    PE : scT tile [128m, 1024n] = cT[mt].T @ bT[:, h]      (K=64, f32r)
    ACT: E[mt]    = exp(scT)  -> SBUF bf16 (no max subtraction needed:
         |scores| <~ 45, exp spans [e^-45, e^45], safely inside bf16 range)
    PE : o2[ct]  += aT[mt, ct].T @ E[mt]   (K=128, bf16, PSUM f32, 2 chunks)
    DVE: Zacc    += E[mt]                  (bf16 column-sum partials)
  per half tail:
    PE : Z = ones.T @ Zacc   (partition reduce, [1, 1024])
    ACT: copy Z PSUM -> SBUF
    DVE: r = reciprocal_approx_fast(Z)     (~18 bits, plenty)
    PE : rb = ones_row outer r             (broadcast 1/Z to 128 partitions)
    DVE: out = o2 * rb + a32 -> DMA

This keeps the PE continuously busy with real work (it ramps to the 2.4GHz
pstate and stays there) instead of ACT-gated with keep-warm filler.

PSUM budget (8 banks): sc [128,1024] = 2, o2 4x[128,512] = 4, z/rb = 2.

Host side pre-arranges every tensor into the exact SBUF layout so each DMA
is one big contiguous-per-partition transfer (few descriptors).
"""

from contextlib import ExitStack

import numpy as np
import ml_dtypes

import concourse.bass as bass
import concourse.tile as tile
from concourse import mybir
from concourse.vector_clock import ScopedClock, VectorClock
from concourse.bass_utils import run_bass_kernel_spmd

F32 = mybir.dt.float32
F32R = mybir.dt.float32r
BF16 = mybir.dt.bfloat16

B, C, S, D = 8, 256, 2048, 64
N_CORES = 8
MT = S // 128       # 16 m-tiles
W = 1024            # n-window (half) width
NH = S // W         # 2 halves
NCT = C // 128      # 2 c-chunks
N_WARM = 6          # PE warmup matmuls (pstate ramp during input DMA)


class PatchedTileContext(tile.TileContext):
    """This walrus build caps sync waits per SP Drain/NoOp at <3; the stock
    TileContext tail drain carries one wait per outstanding semaphore.
    Split them one-per-NOP before a clean drain."""

    def _drain_and_barrier(self, tick_clock, wait_clock):
        gclock = tick_clock.global_clock
        nprocs = len(gclock)
        for proc in range(nprocs):
            tick = gclock[proc]
            if tick <= 0:
                continue
            vec = [0] * nprocs
            vec[proc] = tick
            nop_inst = self.nc.sync.nop(nofuse=True)
            wait_clock.add_sem_waits(
                nop_inst.ins, ScopedClock({None: VectorClock(vec)})
            )
        self.nc.sync.drain()
        self.nc.all_engine_barrier()
        assert self.sems is not None
        popped = self.nc._tile_sem_poison_stack.pop()
        assert popped is self._sem_poison
        self.nc.clear_and_free_semaphores(list(self.sems.allocated().values()))
        self.nc.all_engine_barrier()


def _split_sync_waits_json(raw: bytes, cap: int = 1) -> bytes:
    """This walrus build rejects instructions carrying more than ~1 sync
    wait (setupSyncWait: "Too many sync wait commands").  Rewrite the BIR
    JSON so any instruction keeps at most `cap` waits and the excess move
    to NoOps injected immediately before it in the same engine stream —
    identical semantics, compiler-acceptable encoding."""
    import json

    m = json.loads(raw)
    ctr = 0
    for fn in m["functions"]:
        for bb in fn["blocks"]:
            new_insts = []
            for inst in bb["instructions"]:
                si = inst.get("sync_info") or {}
                ow = si.get("on_wait") or []
                if len(ow) > cap:
                    n_extra = len(ow) - cap
                    for w in ow[:n_extra]:
                        ctr += 1
                        nop = {
                            "engine": inst["engine"],
                            "ins": [],
                            "name": f"I-{90000 + ctr}",
                            "opcode": "NoOp",
                            "outs": [],
                            "sync_info": {"on_update": [], "on_wait": [w]},
                        }
                        if inst.get("debug") is not None:
                            nop["debug"] = inst["debug"]
                        new_insts.append(nop)
                    si["on_wait"] = ow[n_extra:]
                new_insts.append(inst)
            bb["instructions"] = new_insts
    return json.dumps(m).encode()


def build_nc() -> bass.Bass:
    nc = bass.Bass()
    bT = nc.declare_dram_parameter("bT", [D, S], F32R, isOutput=False)
    cT = nc.declare_dram_parameter("cT", [D, S], F32R, isOutput=False)
    # aTb[k, mt, c] = a[mt*128+k, c] of a^T  (bf16, SBUF-exact layout)
    aTb = nc.declare_dram_parameter("aTb", [128, MT * C], BF16, isOutput=False)
    # a32[p, ct, h, nn] = a[ct*128+p, h*W+nn]  (f32, SBUF-exact layout)
    a32 = nc.declare_dram_parameter("a32", [128, NCT * S], F32, isOutput=False)
    # outb[p, ct, h, nn] -> out[ct*128+p, h*W+nn]
    outb = nc.declare_dram_parameter("outb", [128, NCT * S], F32, isOutput=True)

    Exp = mybir.ActivationFunctionType.Exp
    Ln = mybir.ActivationFunctionType.Ln

    with PatchedTileContext(nc) as tc, ExitStack() as ctx:
        # ---------- SBUF pools ----------
        const = ctx.enter_context(tc.tile_pool(name="const", bufs=1))
        inp = ctx.enter_context(tc.tile_pool(name="inp", bufs=1))
        epool = ctx.enter_context(tc.tile_pool(name="epool", bufs=2))
        zpool = ctx.enter_context(tc.tile_pool(name="zpool", bufs=2))
        fin = ctx.enter_context(tc.tile_pool(name="fin", bufs=2))
        outp = ctx.enter_context(tc.tile_pool(name="outp", bufs=4))

        # ---------- PSUM pools (8 banks exactly) ----------
        # scp ring (2 x [128,1024] = 4 banks) double-buffers the score
        # tiles; the per-half tail (z reduce, 1/Z broadcast) borrows the
        # same ring via same-tag allocations, so no extra banks.
        scp = ctx.enter_context(tc.tile_pool(name="scp", bufs=2, space="PSUM"))
        o2p = ctx.enter_context(tc.tile_pool(name="o2p", bufs=1, space="PSUM"))

        # ---------- constants via memset (no DMA dependency) ----------
        onesc = const.tile([128, 1], BF16, tag="onesc")
        nc.vector.memset(onesc, 1.0)
        onesr = const.tile([1, 128], BF16, tag="onesr")
        nc.vector.memset(onesr, 1.0)
        warm_sb = const.tile([128, 512], BF16, tag="warm_sb")
        nc.vector.memset(warm_sb, 1.0)
        kbias = const.tile([128, 1], F32, tag="kbias")
        nc.vector.memset(kbias, -22.0)

        # ---------- input DMAs (contiguous per partition; phase-1 first) ----
        cT_sb = inp.tile([D, S], F32R, tag="cT")
        nc.sync.dma_start(out=cT_sb, in_=cT[:, :])
        bT_sb = inp.tile([D, S], F32R, tag="bT")
        nc.sync.dma_start(out=bT_sb, in_=bT[:, :])
        aT_sb = inp.tile([128, MT, C], BF16, tag="aT")
        aT_r = aTb.rearrange("p (t c) -> p t c", t=MT)
        nc.sync.dma_start(out=aT_sb[:, 0:4, :], in_=aT_r[:, 0:4, :])
        nc.sync.dma_start(out=aT_sb[:, 4:MT, :], in_=aT_r[:, 4:MT, :])
        a32_sb = inp.tile([128, NCT, NH, W], F32, tag="a32")
        nc.sync.dma_start(
            out=a32_sb, in_=a32.rearrange("p (ct h nn) -> p ct h nn", ct=NCT, h=NH)
        )
        out_r = outb.rearrange("p (ct h nn) -> p ct h nn", ct=NCT, h=NH)

        # ---------- PE warmup: ramp pstate while inputs stream in ----------
        warm_ps = o2p.tile([1, 512], F32, name="warm_ps", tag="o2_0_0")
        for _ in range(N_WARM):
            nc.tensor.matmul(
                warm_ps,
                lhsT=onesc[:, 0:1],
                rhs=warm_sb[:, :],
                start=True,
                stop=True,
                skip_group_check=True,
            )

        # ---------- main: two n-halves ----------
        for h in range(NH):
            o2 = [
                o2p.tile([128, 512], F32, name=f"o2_{h}_{ct}_{j}", tag=f"o2_{ct}_{j}")
                for ct in range(NCT)
                for j in range(2)
            ]  # index ct*2+j ; n-chunk j covers half-local [j*512,(j+1)*512)
            zacc = None
            prev_e = None

            # software pipeline: phase-2 of m-tile mt-1 issues AFTER the
            # score matmuls of mt, so the PE streams sc(mt+1) while ACT is
            # still exp-ing sc(mt) (sc is double-buffered in the scp ring).
            for mt in range(MT):
                sc = scp.tile([128, W], F32, name=f"sc{h}_{mt}", tag="sc")
                for j in range(2):
                    nc.tensor.matmul(
                        sc[:, j * 512 : (j + 1) * 512],
                        lhsT=cT_sb[:, mt * 128 : (mt + 1) * 128],
                        rhs=bT_sb[:, h * W + j * 512 : h * W + (j + 1) * 512],
                        start=True,
                        stop=True,
                    )
                # exp(sc - K) -> bf16 SBUF (one big ACT op over both banks).
                # K shifts Z into the ACT Ln table's accurate range
                # [e^-6, e^44] (it breaks above ~e^44); softmax is
                # shift-invariant so K cancels exactly in o2 * (1/Z).
                e = epool.tile([128, W], BF16, name=f"e{h}_{mt}", tag="e")
                nc.scalar.activation(e, sc[:, :], Exp, bias=kbias[:, 0:1])
                if prev_e is not None:
                    for ct in range(NCT):
                        cs = slice(ct * 128, (ct + 1) * 128)
                        for j in range(2):
                            nc.tensor.matmul(
                                o2[ct * 2 + j],
                                lhsT=aT_sb[:, mt - 1, cs],
                                rhs=prev_e[:, j * 512 : (j + 1) * 512],
                                start=(mt == 1),
                                stop=False,
                            )
                # Z partials on DVE (bf16, 4x mode; ping-pong buffers)
                znew = zpool.tile([128, W], BF16, name=f"zacc{h}_{mt}", tag="zacc")
                if mt == 0:
                    nc.vector.tensor_scalar_mul(znew, e, 1.0)
                else:
                    nc.vector.tensor_add(znew, zacc, e)
                zacc = znew
                prev_e = e
            # drain phase-2 for the last m-tile
            for ct in range(NCT):
                cs = slice(ct * 128, (ct + 1) * 128)
                for j in range(2):
                    nc.tensor.matmul(
                        o2[ct * 2 + j],
                        lhsT=aT_sb[:, MT - 1, cs],
                        rhs=prev_e[:, j * 512 : (j + 1) * 512],
                        start=False,
                        stop=True,
                    )

            # ---------- half tail ----------
            # partition-reduce Z into one scp ring slot (2 banks)
            zz = scp.tile([1, W], F32, name=f"zz{h}", tag="sc")
            for j in range(2):
                nc.tensor.matmul(
                    zz[:, j * 512 : (j + 1) * 512],
                    lhsT=onesc[:, 0:1],
                    rhs=zacc[:, j * 512 : (j + 1) * 512],
                    start=True,
                    stop=True,
                )
            # r = exp(-ln(Z)) on ACT (ln/exp/copy share one act table set;
            # custom-DVE reciprocal ops don't compile on this walrus build
            # and InstReciprocal costs ~6.3ns/elem).  ACT reads Z straight
            # from PSUM and writes bf16 so the broadcast matmul can consume
            # it; rb is then copied to SBUF because the verifier allows only
            # one PSUM operand per DVE TensorTensor.
            l_sb = fin.tile([1, W], F32, name=f"lsb{h}", tag="l_sb")
            nc.scalar.activation(l_sb, zz[:, :], Ln)
            r_sb = fin.tile([1, W], BF16, name=f"rsb{h}", tag="r_sb")
            nc.scalar.activation(r_sb, l_sb, Exp, scale=-1.0)
            rb = scp.tile([128, W], F32, name=f"rb{h}", tag="sc")
            rb_sb = fin.tile([128, W], F32, name=f"rbsb{h}", tag="rb_sb")
            for j in range(2):
                nc.tensor.matmul(
                    rb[:, j * 512 : (j + 1) * 512],
                    lhsT=onesr[:, :],
                    rhs=r_sb[:, j * 512 : (j + 1) * 512],
                    start=True,
                    stop=True,
                )
                nc.scalar.copy(
                    rb_sb[:, j * 512 : (j + 1) * 512],
                    rb[:, j * 512 : (j + 1) * 512],
                )
            # normalize + residual, store
            for ct in range(NCT):
                for j in range(2):
                    t1 = fin.tile([128, 512], F32, tag=f"t1_{j}")
                    nc.vector.tensor_mul(
                        t1, o2[ct * 2 + j], rb_sb[:, j * 512 : (j + 1) * 512]
                    )
                    o_sb = outp.tile([128, 512], F32, tag=f"o_sb{j}")
                    nc.vector.tensor_add(o_sb, t1, a32_sb[:, ct, h, j * 512 : (j + 1) * 512])
                    nc.sync.dma_start(
                        out=out_r[:, ct, h, j * 512 : (j + 1) * 512], in_=o_sb
                    )

    orig_to_json_bytes = nc.to_json_bytes

    def to_json_bytes():
        return _split_sync_waits_json(orig_to_json_bytes())

    nc.to_json_bytes = to_json_bytes
    return nc


_NC_CACHE = None


def _get_nc():
    global _NC_CACHE
    if _NC_CACHE is None:
        _NC_CACHE = build_nc()
    return _NC_CACHE


def kernel(a, b, c, **run_kwargs):
    """a: [8, 256, 2048] f32, b: [8, 2048, 64] f32, c: [8, 2048, 64] f32
    -> [8, 256, 2048] f32"""
    a = np.asarray(a, dtype=np.float32)
    b = np.asarray(b, dtype=np.float32)
    c = np.asarray(c, dtype=np.float32)
    in_maps = []
    for i in range(N_CORES):
        aT = np.ascontiguousarray(a[i].T)  # [S, C]
        aTb = (
            aT.reshape(MT, 128, C)
            .transpose(1, 0, 2)
            .reshape(128, MT * C)
            .astype(ml_dtypes.bfloat16)
        )
        a32 = np.ascontiguousarray(
            a[i].reshape(NCT, 128, NH, W).transpose(1, 0, 2, 3).reshape(128, NCT * S)
        )
        in_maps.append(
            {
                "bT": np.ascontiguousarray(b[i].T),
                "cT": np.ascontiguousarray(c[i].T),
                "aTb": aTb,
                "a32": a32,
            }
        )
    res = run_bass_kernel_spmd(_get_nc(), in_maps, list(range(N_CORES)), **run_kwargs)
    out = np.stack(
        [
            np.asarray(res.results[i]["outb"])
            .reshape(128, NCT, NH, W)
            .transpose(1, 0, 2, 3)
            .reshape(C, S)
            for i in range(N_CORES)
        ]
    )
    if run_kwargs:
        kernel.last_result = res
    return out.astype(np.float32)



# revision 2
# speedup vs baseline: 1.2143x; 1.2143x over previous
"""Trainium2 Bass kernel for nn_CrossLayer (B=8, C=256, S=2048, D=64).

Reference computation (per batch b):
    scores = b_i @ c_i^T               [S, S]
    mid    = softmax(scores, axis=-1)  row softmax over m
    out    = a_i @ mid^T + a_i         [C, S]

Strategy: data-parallel over batch - one batch per NeuronCore (8 cores).
Everything is computed in the "column" layout scoresT[m, n] so the softmax
contraction axis m sits on SBUF partitions, which is what the second matmul
needs.  The n axis is split into two halves of 1024.

v2 design (vs the f32r baseline):
  * Phase-1 inputs are fp16 (precision measured: 2.9e-3 scale-rel absmax
    vs 2.2e-3 for f32 - fp16's 11-bit mantissa keeps score error ~0.005
    absolute, harmless through exp).  fp16 streams 1 col/cycle vs ~2 for
    f32r: phase-1 matmul time halves.
  * K=64 < 128 wastes half the PE array, so phase-1 m-tiles are processed
    in PAIRS with tile_position row-packing: m-tile A contracts on array
    rows 0-63 (cT/bT copy in partitions 0-63), m-tile B on rows 64-127
    (duplicate copy in partitions 64-127).  The two matmul streams run
    concurrently in different row-groups: ~2x phase-1 throughput.
  * Phase-2 (o2[c,n] += aT[m,c].T @ E[m,n]) is serialized over the two
    c-chunks: ct0 accumulates in-loop (1 pair behind phase-1), ct1 MMs
    trail 2 pairs behind, reading E tiles parked in SBUF.  This keeps o2
    at 2 PSUM banks per live c-chunk so PSUM = sc ring (2x2 banks) + o2
    ring (2x2 banks) exactly.
  * 1/Z: rbZ = allones[128,128].T @ zacc broadcasts the column sum Z to
    all 128 partitions in one matmul, then ACT does Ln + Exp(-x) on
    [128,1024] (ACT cost depends only on free-dim elems/lane, so the
    broadcast is free) -> r_sb in SBUF, no PSUM->SBUF copy, no [1,N] ops.
  * Final normalize runs in bf16 (t1, residual, output): DVE 16-bit rate
    and half the output DMA bytes.  Measured total error 5.7e-3 << 2e-2.

PSUM budget (8 banks): sc ring 2 x [128,1024] = 4, o2 ring 2 x [128,1024]
= 4.  The per-half rbZ tile borrows an sc ring slot.
"""

from contextlib import ExitStack

import numpy as np
import ml_dtypes

import concourse.bass as bass
import concourse.tile as tile
from concourse import mybir
from concourse.vector_clock import ScopedClock, VectorClock
from concourse.bass_utils import run_bass_kernel_spmd

F32 = mybir.dt.float32
FP16 = mybir.dt.float16
BF16 = mybir.dt.bfloat16

B, C, S, D = 8, 256, 2048, 64
N_CORES = 8
MT = S // 128       # 16 m-tiles
W = 1024            # n-window (half) width
NH = S // W         # 2 halves
NCT = C // 128      # 2 c-chunks
NP = MT // 2        # 8 m-tile pairs per half
N_WARM = 8          # PE warmup matmuls (pstate ramp during input DMA)


class PatchedTileContext(tile.TileContext):
    """This walrus build caps sync waits per SP Drain/NoOp at <3; the stock
    TileContext tail drain carries one wait per outstanding semaphore.
    Split them one-per-NOP before a clean drain."""

    def _drain_and_barrier(self, tick_clock, wait_clock):
        gclock = tick_clock.global_clock
        nprocs = len(gclock)
        for proc in range(nprocs):
            tick = gclock[proc]
            if tick <= 0:
                continue
            vec = [0] * nprocs
            vec[proc] = tick
            nop_inst = self.nc.sync.nop(nofuse=True)
            wait_clock.add_sem_waits(
                nop_inst.ins, ScopedClock({None: VectorClock(vec)})
            )
        self.nc.sync.drain()
        self.nc.all_engine_barrier()
        assert self.sems is not None
        popped = self.nc._tile_sem_poison_stack.pop()
        assert popped is self._sem_poison
        self.nc.clear_and_free_semaphores(list(self.sems.allocated().values()))
        self.nc.all_engine_barrier()


def _split_sync_waits_json(raw: bytes, cap: int = 1) -> bytes:
    """This walrus build rejects instructions carrying more than ~1 sync
    wait (setupSyncWait: "Too many sync wait commands").  Rewrite the BIR
    JSON so any instruction keeps at most `cap` waits and the excess move
    to NoOps injected immediately before it in the same engine stream -
    identical semantics, compiler-acceptable encoding."""
    import json

    m = json.loads(raw)
    ctr = 0
    for fn in m["functions"]:
        for bb in fn["blocks"]:
            new_insts = []
            for inst in bb["instructions"]:
                si = inst.get("sync_info") or {}
                ow = si.get("on_wait") or []
                if len(ow) > cap:
                    n_extra = len(ow) - cap
                    for w in ow[:n_extra]:
                        ctr += 1
                        nop = {
                            "engine": inst["engine"],
                            "ins": [],
                            "name": f"I-{90000 + ctr}",
                            "opcode": "NoOp",
                            "outs": [],
                            "sync_info": {"on_update": [], "on_wait": [w]},
                        }
                        if inst.get("debug") is not None:
                            nop["debug"] = inst["debug"]
                        new_insts.append(nop)
                    si["on_wait"] = ow[n_extra:]
                new_insts.append(inst)
            bb["instructions"] = new_insts
    return json.dumps(m).encode()


def build_nc() -> bass.Bass:
    nc = bass.Bass()
    # bTd/cTd[k, :] duplicated: rows 0-63 and 64-127 hold the same [D, S]
    # transposed tensor, so packed matmuls can contract on either array half
    bTd = nc.declare_dram_parameter("bTd", [128, S], FP16, isOutput=False)
    cTd = nc.declare_dram_parameter("cTd", [128, S], FP16, isOutput=False)
    # aTb[k, mt, c] = a[mt*128+k, c] of a^T  (bf16, SBUF-exact layout)
    aTb = nc.declare_dram_parameter("aTb", [128, MT * C], BF16, isOutput=False)
    # ar[p, ct, h, nn] = a[ct*128+p, h*W+nn]  (bf16 residual, SBUF layout)
    ar = nc.declare_dram_parameter("ar", [128, NCT * S], BF16, isOutput=False)
    # outb[p, ct, h, nn] -> out[ct*128+p, h*W+nn]  (bf16)
    outb = nc.declare_dram_parameter("outb", [128, NCT * S], BF16, isOutput=True)

    Exp = mybir.ActivationFunctionType.Exp
    Ln = mybir.ActivationFunctionType.Ln

    with PatchedTileContext(nc) as tc, ExitStack() as ctx:
        # ---------- SBUF pools ----------
        const = ctx.enter_context(tc.tile_pool(name="const", bufs=1))
        inp = ctx.enter_context(tc.tile_pool(name="inp", bufs=1))
        epool = ctx.enter_context(tc.tile_pool(name="epool", bufs=8))
        zpool = ctx.enter_context(tc.tile_pool(name="zpool", bufs=2))
        fin = ctx.enter_context(tc.tile_pool(name="fin", bufs=2))
        outp = ctx.enter_context(tc.tile_pool(name="outp", bufs=4))

        # ---------- PSUM pools (8 banks exactly) ----------
        scp = ctx.enter_context(tc.tile_pool(name="scp", bufs=2, space="PSUM"))
        o2p = ctx.enter_context(tc.tile_pool(name="o2p", bufs=2, space="PSUM"))

        # ---------- constants via memset (no DMA dependency) ----------
        onesc = const.tile([128, 1], BF16, tag="onesc")
        nc.vector.memset(onesc, 1.0)
        allones = const.tile([128, 128], BF16, tag="allones")
        nc.vector.memset(allones, 1.0)
        warm_sb = const.tile([128, 512], BF16, tag="warm_sb")
        nc.vector.memset(warm_sb, 1.0)
        kbias = const.tile([128, 1], F32, tag="kbias")
        nc.vector.memset(kbias, -22.0)

        # ---------- input DMAs (phase-1 operands first, split for overlap) --
        cT_sb = inp.tile([128, S], FP16, tag="cT")
        nc.sync.dma_start(out=cT_sb[:, 0:512], in_=cTd[:, 0:512])
        bT_sb = inp.tile([128, S], FP16, tag="bT")
        nc.sync.dma_start(out=bT_sb[:, 0:W], in_=bTd[:, 0:W])
        nc.sync.dma_start(out=cT_sb[:, 512:S], in_=cTd[:, 512:S])
        nc.sync.dma_start(out=bT_sb[:, W:S], in_=bTd[:, W:S])
        aT_sb = inp.tile([128, MT, C], BF16, tag="aT")
        aT_r = aTb.rearrange("p (t c) -> p t c", t=MT)
        nc.sync.dma_start(out=aT_sb[:, 0:4, :], in_=aT_r[:, 0:4, :])
        nc.sync.dma_start(out=aT_sb[:, 4:MT, :], in_=aT_r[:, 4:MT, :])
        ar_sb = inp.tile([128, NCT, NH, W], BF16, tag="ar")
        ar_r = ar.rearrange("p (ct h nn) -> p ct h nn", ct=NCT, h=NH)
        nc.sync.dma_start(out=ar_sb[:, 0, :, :], in_=ar_r[:, 0, :, :])
        nc.sync.dma_start(out=ar_sb[:, 1, :, :], in_=ar_r[:, 1, :, :])
        out_r = outb.rearrange("p (ct h nn) -> p ct h nn", ct=NCT, h=NH)

        # ---------- PE warmup: ramp pstate while inputs stream in ----------
        warm_ps = o2p.tile([1, 512], F32, name="warm_ps", tag="o2")
        for _ in range(N_WARM):
            nc.tensor.matmul(
                warm_ps,
                lhsT=onesc[:, 0:1],
                rhs=warm_sb[:, :],
                start=True,
                stop=True,
                skip_group_check=True,
            )

        # ---------- main: two n-halves ----------
        for h in range(NH):
            o2 = [
                o2p.tile([128, W], F32, name=f"o2_{h}_{ct}", tag="o2")
                for ct in range(NCT)
            ]
            E = {}
            zacc = None
            hw0 = h * W

            def ct_mms(ct, pr):
                # phase-2 matmuls for both m-tiles of pair `pr`, c-chunk ct
                for mt in (2 * pr, 2 * pr + 1):
                    for j in range(2):
                        nc.tensor.matmul(
                            o2[ct][:, j * 512 : (j + 1) * 512],
                            lhsT=aT_sb[:, mt, ct * 128 : (ct + 1) * 128],
                            rhs=E[mt][:, j * 512 : (j + 1) * 512],
                            start=(mt == 0),
                            stop=(mt == MT - 1),
                        )

            for p in range(NP):
                mtA, mtB = 2 * p, 2 * p + 1
                scA = scp.tile([128, W], F32, name=f"scA{h}_{p}", tag="sc")
                scB = scp.tile([128, W], F32, name=f"scB{h}_{p}", tag="sc")
                # packed score matmuls: A on array rows 0-63, B on 64-127.
                # Issue order A0,B0,A1,B1: B's LDWEIGHTS targets the other
                # row-group so each B matmul runs concurrently with its A.
                for j in range(2):
                    nc.tensor.matmul(
                        scA[:, j * 512 : (j + 1) * 512],
                        lhsT=cT_sb[0:64, mtA * 128 : (mtA + 1) * 128],
                        rhs=bT_sb[0:64, hw0 + j * 512 : hw0 + (j + 1) * 512],
                        start=True,
                        stop=True,
                        tile_position=(0, 0),
                    )
                    nc.tensor.matmul(
                        scB[:, j * 512 : (j + 1) * 512],
                        lhsT=cT_sb[64:128, mtB * 128 : (mtB + 1) * 128],
                        rhs=bT_sb[64:128, hw0 + j * 512 : hw0 + (j + 1) * 512],
                        start=True,
                        stop=True,
                        tile_position=(64, 0),
                    )
                # phase-2 fillers keep the PE busy while ACT exps this pair
                if p >= 1:
                    ct_mms(0, p - 1)
                if p >= 2:
                    ct_mms(1, p - 2)
                # exp(sc - K) -> bf16 SBUF.  K=-22 keeps Z inside the ACT Ln
                # table's accurate range; softmax shift-invariance cancels it.
                eA = epool.tile([128, W], BF16, name=f"e{h}_{mtA}", tag="e")
                nc.scalar.activation(eA, scA[:, :], Exp, bias=kbias[:, 0:1])
                eB = epool.tile([128, W], BF16, name=f"e{h}_{mtB}", tag="e")
                nc.scalar.activation(eB, scB[:, :], Exp, bias=kbias[:, 0:1])
                E[mtA], E[mtB] = eA, eB
                # Z partials on DVE (bf16 ping-pong)
                for mt, e in ((mtA, eA), (mtB, eB)):
                    znew = zpool.tile([128, W], BF16, name=f"z{h}_{mt}", tag="zacc")
                    if mt == 0:
                        nc.vector.tensor_scalar_mul(znew, e, 1.0)
                    else:
                        nc.vector.tensor_add(znew, zacc, e)
                    zacc = znew

            # drain phase-2
            ct_mms(0, NP - 1)
            ct_mms(1, NP - 2)
            ct_mms(1, NP - 1)

            # ---------- half tail ----------
            # rbZ[p, n] = Z[n] for all p: all-ones stationary broadcasts the
            # partition-sum to every partition in one matmul.
            rbZ = scp.tile([128, W], F32, name=f"rbZ{h}", tag="sc")
            for j in range(2):
                nc.tensor.matmul(
                    rbZ[:, j * 512 : (j + 1) * 512],
                    lhsT=allones[:, :],
                    rhs=zacc[:, j * 512 : (j + 1) * 512],
                    start=True,
                    stop=True,
                )
            # r = exp(-ln(Z)) on ACT; [128,W] costs the same as [1,W] and
            # lands broadcast-ready in SBUF (ln/exp share one table set).
            lnt = fin.tile([128, W], F32, name=f"ln{h}", tag="lnt")
            nc.scalar.activation(lnt, rbZ[:, :], Ln)
            rsb = fin.tile([128, W], F32, name=f"r{h}", tag="rsb")
            nc.scalar.activation(rsb, lnt, Exp, scale=-1.0)
            # normalize + residual (bf16), store
            for ct in range(NCT):
                for j in range(2):
                    t1 = fin.tile(
                        [128, 512], BF16, name=f"t1_{h}_{ct}_{j}", tag=f"t1_{j}"
                    )
                    nc.vector.tensor_mul(
                        t1, o2[ct][:, j * 512 : (j + 1) * 512],
                        rsb[:, j * 512 : (j + 1) * 512],
                    )
                    o_sb = outp.tile(
                        [128, 512], BF16, name=f"o_{h}_{ct}_{j}", tag=f"o_{j}"
                    )
                    nc.vector.tensor_add(
                        o_sb, t1, ar_sb[:, ct, h, j * 512 : (j + 1) * 512]
                    )
                    nc.sync.dma_start(
                        out=out_r[:, ct, h, j * 512 : (j + 1) * 512], in_=o_sb
                    )

    orig_to_json_bytes = nc.to_json_bytes

    def to_json_bytes():
        return _split_sync_waits_json(orig_to_json_bytes())

    nc.to_json_bytes = to_json_bytes
    return nc


_NC_CACHE = None


def _get_nc():
    global _NC_CACHE
    if _NC_CACHE is None:
        _NC_CACHE = build_nc()
    return _NC_CACHE


def kernel(a, b, c, **run_kwargs):
    """a: [8, 256, 2048] f32, b: [8, 2048, 64] f32, c: [8, 2048, 64] f32
    -> [8, 256, 2048] f32"""
    a = np.asarray(a, dtype=np.float32)
    b = np.asarray(b, dtype=np.float32)
    c = np.asarray(c, dtype=np.float32)
    in_maps = []
    for i in range(N_CORES):
        bT = np.ascontiguousarray(b[i].T)  # [D, S]
        cT = np.ascontiguousarray(c[i].T)
        bTd = np.concatenate([bT, bT], axis=0).astype(np.float16)
        cTd = np.concatenate([cT, cT], axis=0).astype(np.float16)
        aT = np.ascontiguousarray(a[i].T)  # [S, C]
        aTb = (
            aT.reshape(MT, 128, C)
            .transpose(1, 0, 2)
            .reshape(128, MT * C)
            .astype(ml_dtypes.bfloat16)
        )
        ar = (
            a[i]
            .reshape(NCT, 128, NH, W)
            .transpose(1, 0, 2, 3)
            .reshape(128, NCT * S)
            .astype(ml_dtypes.bfloat16)
        )
        in_maps.append({"bTd": bTd, "cTd": cTd, "aTb": aTb, "ar": ar})
    res = run_bass_kernel_spmd(_get_nc(), in_maps, list(range(N_CORES)), **run_kwargs)
    out = np.stack(
        [
            np.asarray(res.results[i]["outb"])
            .astype(np.float32)
            .reshape(128, NCT, NH, W)
            .transpose(1, 0, 2, 3)
            .reshape(C, S)
            for i in range(N_CORES)
        ]
    )
    if run_kwargs:
        kernel.last_result = res
    return out
